# revision 27
# baseline (speedup 1.0000x reference)
import sys, os, time
sys.path.insert(0, '/opt/trn_rl_repo')
os.environ.setdefault("JAX_PLATFORMS", "")

import numpy as np
import ml_dtypes

import concourse.bass as bass
import concourse.bacc as bacc
import concourse.bass_isa as bass_isa
import concourse.mybir as mybir
import concourse.tile as tile
from concourse import bass2jax

BF = ml_dtypes.bfloat16
B, S, D, H, DH = 4, 2048, 2048, 16, 128
INV_SQRT_DH = 1.0 / np.sqrt(128.0)

_CACHE = {}
LAST_EXEC_NS = None


def _build(stages="ABC", reps=1, loop_n=1):
    nc = bacc.Bacc("TRN2", target_bir_lowering=False, debug=False, num_devices=8)
    f32, bf16 = mybir.dt.float32, mybir.dt.bfloat16
    x_ap = nc.dram_tensor("x_img", (128, 32768), bf16, kind="ExternalInput").ap()
    wqk_ap = nc.dram_tensor("wqk_img", (128, 32768), bf16, kind="ExternalInput").ap()
    wv_ap = nc.dram_tensor("wv_img", (128, 16384), bf16, kind="ExternalInput").ap()
    wout_ap = nc.dram_tensor("wout_img", (128, 16384), bf16, kind="ExternalInput").ap()
    bqk_ap = nc.dram_tensor("bqk", (128, 16), f32, kind="ExternalInput").ap()
    bv_ap = nc.dram_tensor("bias_v", (128, 1024), f32, kind="ExternalInput").ap()
    mask_ap = nc.dram_tensor("masks", (128, 2048), bf16, kind="ExternalInput").ap()
    onc_ap = nc.dram_tensor("ones_col", (128, 1), bf16, kind="ExternalInput").ap()
    onr_ap = nc.dram_tensor("ones_row", (1, 128), bf16, kind="ExternalInput").ap()
    out_ap = nc.dram_tensor("out", (2048, 2048), f32, kind="ExternalOutput").ap()

    with tile.TileContext(nc) as tc:
        with tc.tile_pool(name="persist", bufs=1) as pp:
            qk_sb = pp.tile([128, 32768], bf16)    # [dh, ob*2048+s], ob=2hl: Q_h^T, 2hl+1: K_h^T
            v_sb = pp.tile([128, 16384], bf16)     # [s%128, st*1024 + hl*128 + dh]
            bqk_sb = pp.tile([128, 16], f32)
            bv_sb = pp.tile([128, 1024], f32)
            ones_c = pp.tile([128, 1], bf16)
            ones_r = pp.tile([1, 128], bf16)
            nc.sync.dma_start(bqk_sb, bqk_ap)
            nc.sync.dma_start(bv_sb, bv_ap)
            nc.sync.dma_start(ones_c, onc_ap)
            nc.sync.dma_start(ones_r, onr_ap)

            def _kernel_body():
                # ---------------- Stage A: QKV projections ----------------
                with tc.tile_pool(name="xTp", bufs=1) as xTp:
                    xT = xTp.tile([128, 32768], bf16)  # [d%128, dt*2048 + s]
                    for dt in range(16 if "A" in stages else 0):
                        nc.sync.dma_start(xT[:, dt*2048:(dt+1)*2048],
                                          x_ap[:, dt*2048:(dt+1)*2048])

                    # A-V first: v_sb = x @ Wv^T + bv   (psum[s, ov])
                    with (
                        tc.tile_pool(name="wvp", bufs=1) as wvp,
                        tc.tile_pool(name="psV", bufs=2, space="PSUM") as psV,
                    ):
                        for oc in range(2 if "A" in stages else 0):
                            wv_oc = wvp.tile([128, 8192], bf16)  # [d%128, dt*512 + ov%512]
                            for dt in range(16):
                                nc.sync.dma_start(
                                    wv_oc[:, dt*512:(dt+1)*512],
                                    wv_ap[:, dt*1024 + oc*512: dt*1024 + (oc+1)*512])
                            for st in range(16):
                                ps = psV.tile([128, 512], f32)
                                for dt in range(16):
                                    nc.tensor.matmul(
                                        ps,
                                        xT[:, dt*2048 + st*128: dt*2048 + (st+1)*128],
                                        wv_oc[:, dt*512:(dt+1)*512],
                                        start=(dt == 0), stop=(dt == 15))
                                nc.vector.tensor_tensor(
                                    v_sb[:, st*1024 + oc*512: st*1024 + (oc+1)*512],
                                    ps, bv_sb[:, oc*512:(oc+1)*512], mybir.AluOpType.add)

                    # A-QK: qk_sb = Wqk @ x^T + b      (psum[o, s])
                    # paired [128,1024] psum tiles, bias-add on ScalarE
                    # dt-outer / sc-inner: the slab weight block stays the
                    # stationary operand across 4 matmuls (LDWEIGHTS reuse);
                    # 4 accumulation groups live in 2 paired psum tiles.
                    with (
                        tc.tile_pool(name="wqkp", bufs=2) as wqkp,
                        tc.tile_pool(name="psQ", bufs=3, space="PSUM") as psQ,
                    ):
                        for ob in range(16 if "A" in stages else 0):
                            slab = wqkp.tile([128, 2048], bf16)  # [d%128, dt*128 + o]
                            nc.sync.dma_start(slab, wqk_ap[:, ob*2048:(ob+1)*2048])
                            pq = [psQ.tile([128, 1024], f32, tag="psq",
                                           name=f"pq{j}")
                                  for j in range(2)]
                            for dt in range(16):
                                for sc in range(4):
                                    nc.tensor.matmul(
                                        pq[sc // 2][:, (sc % 2)*512:(sc % 2 + 1)*512],
                                        slab[:, dt*128:(dt+1)*128],
                                        xT[:, dt*2048 + sc*512: dt*2048 + (sc+1)*512],
                                        start=(dt == 0), stop=(dt == 15),
                                        skip_group_check=True)
                            for sp in range(2):
                                nc.scalar.activation(
                                    qk_sb[:, ob*2048 + sp*1024: ob*2048 + (sp+1)*1024],
                                    pq[sp], mybir.ActivationFunctionType.Identity,
                                    bias=bqk_sb[:, ob:ob+1])

                # ---------------- Stage B: attention ----------------
                with tc.tile_pool(name="bstat", bufs=1) as bstat:
                    masks = bstat.tile([128, 2048], bf16)
                    ctxT = bstat.tile([128, 16384], bf16)  # [dh, hl*2048 + q]
                    wout = bstat.tile([128, 16384], bf16)  # [dh, hl*2048 + od]
                    if "B" in stages:
                        nc.sync.dma_start(masks, mask_ap)
                    for i in range(8 if "C" in stages else 0):
                        nc.sync.dma_start(wout[:, i*2048:(i+1)*2048],
                                          wout_ap[:, i*2048:(i+1)*2048])

                    with (
                        tc.tile_pool(name="expp", bufs=6) as expp,
                        tc.tile_pool(name="accp", bufs=2) as accp,
                        tc.tile_pool(name="recp", bufs=2) as recp,
                        tc.tile_pool(name="bcp", bufs=2) as bcp,
                        tc.tile_pool(name="psS", bufs=2, space="PSUM") as psS,
                        tc.tile_pool(name="psC", bufs=2, space="PSUM") as psC,
                        tc.tile_pool(name="psD", bufs=1, space="PSUM") as psD,
                        tc.tile_pool(name="psB", bufs=1, space="PSUM") as psB,
                    ):
                        # normalization tail of group g is deferred into group
                        # g+1's score stream so the PE never waits on the
                        # DVE/ACT round-trip (dn -> recip -> bc -> mult).
                        def make_tail(hl, ic, cp, acc_bf):
                            def tail():
                                dn = psD.tile([1, 512], f32, name="dn")
                                for h2 in range(2):
                                    nc.tensor.matmul(
                                        dn, ones_c, acc_bf[:, h2*512:(h2+1)*512],
                                        start=(h2 == 0), stop=(h2 == 1),
                                        skip_group_check=True)
                                rec = recp.tile([1, 512], bf16, name="rec")
                                with nc.allow_low_precision(
                                        reason="softmax denom recip in bf16"):
                                    nc.vector.reciprocal(rec, dn)
                                bc = psB.tile([128, 512], f32, name="bc")
                                nc.tensor.matmul(bc, ones_r, rec, start=True,
                                                 stop=True, skip_group_check=True)
                                bc_sb = bcp.tile([128, 512], f32, name="bc_sb")
                                nc.scalar.activation(
                                    bc_sb, bc, mybir.ActivationFunctionType.Copy)
                                nc.vector.tensor_tensor(
                                    ctxT[:, hl*2048 + ic*512: hl*2048 + (ic+1)*512],
                                    cp, bc_sb, mybir.AluOpType.mult)
                            return tail

                        tail_prev = None
                        for hl in range(8 if "B" in stages else 0):
                            qb = (2*hl) * 2048
                            kb = (2*hl+1) * 2048
                            for ic in range(4):
                                njt = 4*ic + 4
                                njp = njt // 2
                                cp = psC.tile([128, 512], f32)
                                acc = accp.tile([128, 1024], bf16)
                                pend = []

                                def flush_ctx():
                                    jp_, ex_ = pend.pop(0)
                                    for h2 in range(2):
                                        jt_ = 2*jp_ + h2
                                        nc.tensor.matmul(
                                            cp,
                                            v_sb[:, jt_*1024 + hl*128: jt_*1024 + (hl+1)*128],
                                            ex_[:, h2*512:(h2+1)*512],
                                            start=(jt_ == 0), stop=(jt_ == njt-1),
                                            skip_group_check=True)

                                for jp in range(njp):
                                    sc = psS.tile([128, 1024], f32)
                                    for h2 in range(2):
                                        jt = 2*jp + h2
                                        nc.tensor.matmul(
                                            sc[:, h2*512:(h2+1)*512],
                                            qk_sb[:, kb + jt*128: kb + (jt+1)*128],
                                            qk_sb[:, qb + ic*512: qb + (ic+1)*512],
                                            start=True, stop=True, skip_group_check=True)
                                    ex = expp.tile([128, 1024], bf16)
                                    nc.scalar.activation(ex, sc,
                                                         mybir.ActivationFunctionType.Exp)
                                    # causal mask applied AFTER exp as a 0/1
                                    # multiply on the SBUF tile, so the psum
                                    # slot recycles at exp time, off the
                                    # score-stream critical chain.
                                    t0 = 2*jp - 4*ic
                                    if t0 >= 0:
                                        nc.vector.tensor_tensor(
                                            ex, ex, masks[:, t0*512:(t0+2)*512],
                                            mybir.AluOpType.mult)
                                    if jp == 0:
                                        nc.vector.tensor_copy(acc, ex)
                                    else:
                                        nc.vector.tensor_tensor(acc, acc, ex,
                                                                mybir.AluOpType.add)
                                    pend.append((jp, ex))
                                    if jp == 0 and tail_prev is not None:
                                        tail_prev()
                                        tail_prev = None
                                    if len(pend) > 3:
                                        flush_ctx()
                                while pend:
                                    flush_ctx()
                                tail_prev = make_tail(hl, ic, cp, acc)
                        if tail_prev is not None:
                            tail_prev()
                            tail_prev = None

                    # ---------------- Stage C: output projection ----------------
                    # hl-outer / dok-inner: the ctxT block stays stationary
                    # across 4 matmuls; 4 accumulation groups in one 4-bank
                    # psum tile per s-row block.
                    with (
                        tc.tile_pool(name="psO", bufs=2, space="PSUM") as psO,
                        tc.tile_pool(name="outp", bufs=2) as outp,
                    ):
                        for st in range(16 if "C" in stages else 0):
                            po = psO.tile([128, 2048], f32)
                            for hl in range(8):
                                for dok in range(4):
                                    nc.tensor.matmul(
                                        po[:, dok*512:(dok+1)*512],
                                        ctxT[:, hl*2048 + st*128: hl*2048 + (st+1)*128],
                                        wout[:, hl*2048 + dok*512: hl*2048 + (dok+1)*512],
                                        start=(hl == 0), stop=(hl == 7),
                                        skip_group_check=True)
                            ob_sb = outp.tile([128, 2048], f32)
                            if st % 2 == 0:
                                nc.vector.tensor_copy(ob_sb, po)
                            else:
                                nc.scalar.activation(
                                    ob_sb, po, mybir.ActivationFunctionType.Copy)
                            nc.sync.dma_start(
                                out_ap[st*128:(st+1)*128, :], ob_sb)

            if loop_n > 1:
                with tc.For_i(0, loop_n, 1):
                    for _rep in range(reps):
                        _kernel_body()
            else:
                for _rep in range(reps):
                    _kernel_body()

    nc.compile()
    return nc


def _prep_inputs(x, attn_mask, w_qkv, b_qkv, w_out, b_out):
    isd = INV_SQRT_DH
    x_imgs = []
    for b in range(4):
        x_imgs.append(np.ascontiguousarray(
            x[b].reshape(2048, 16, 128).transpose(2, 1, 0).reshape(128, 32768)
        ).astype(BF))

    jj = np.arange(128)[:, None]
    ii = np.arange(512)[None, :]
    mblocks = []
    for t in range(4):
        keep = attn_mask[ii, t*128 + jj] != 0
        mblocks.append(np.where(keep, 1.0, 0.0))
    masks = np.concatenate(mblocks, axis=1).astype(BF)

    ones_col = np.ones((128, 1), dtype=BF)
    ones_row = np.ones((1, 128), dtype=BF)

    per_g = []
    for g in range(2):
        rows = np.empty((2048, 2048), np.float32)
        bqk = np.empty((128, 16), np.float32)
        wv_rows = np.empty((1024, 2048), np.float32)
        bv = np.empty(1024, np.float32)
        for hl in range(8):
            h = 8*g + hl
            rows[(2*hl)*128:(2*hl+1)*128] = w_qkv[384*h: 384*h+128] * isd
            rows[(2*hl+1)*128:(2*hl+2)*128] = w_qkv[384*h+128: 384*h+256]
            bqk[:, 2*hl] = b_qkv[384*h: 384*h+128] * isd
            bqk[:, 2*hl+1] = b_qkv[384*h+128: 384*h+256]
            wv_rows[hl*128:(hl+1)*128] = w_qkv[384*h+256: 384*h+384]
            bv[hl*128:(hl+1)*128] = b_qkv[384*h+256: 384*h+384]
        wqk_img = np.ascontiguousarray(
            rows.reshape(16, 128, 16, 128).transpose(3, 0, 2, 1).reshape(128, 32768)
        ).astype(BF)
        wv_img = np.ascontiguousarray(
            wv_rows.reshape(1024, 16, 128).transpose(2, 1, 0).reshape(128, 16384)
        ).astype(BF)
        wout_img = np.ascontiguousarray(
            w_out[:, 1024*g: 1024*(g+1)].reshape(2048, 8, 128)
            .transpose(2, 1, 0).reshape(128, 16384)
        ).astype(BF)
        bias_v = np.ascontiguousarray(
            np.broadcast_to(bv[None, :], (128, 1024))).astype(np.float32)
        per_g.append((wqk_img, wv_img, wout_img, bqk, bias_v))

    in_maps = []
    for c in range(8):
        b, g = c // 2, c % 2
        wqk_img, wv_img, wout_img, bqk, bias_v = per_g[g]
        in_maps.append({
            "x_img": x_imgs[b],
            "wqk_img": wqk_img,
            "wv_img": wv_img,
            "wout_img": wout_img,
            "bqk": bqk,
            "bias_v": bias_v,
            "masks": masks,
            "ones_col": ones_col,
            "ones_row": ones_row,
        })
    return in_maps


N_CORES = 8


def _get_runner():
    """Build nc + jitted shard_map executable once per process."""
    if "runner" in _CACHE:
        return _CACHE["runner"]
    _CACHE["runner"] = _make_runner(_build())
    return _CACHE["runner"]


def _make_runner(nc):
    import jax
    from jax.sharding import Mesh, PartitionSpec
    from jax.experimental.shard_map import shard_map

    bass2jax.install_neuronx_cc_hook()

    partition_name = (nc.partition_id_tensor.name
                      if nc.partition_id_tensor else None)
    in_names, out_names, out_avals, zero_outs = [], [], [], []
    for alloc in nc.m.functions[0].allocations:
        if not isinstance(alloc, mybir.MemoryLocationSet):
            continue
        name = alloc.memorylocations[0].name
        if alloc.kind == "ExternalInput":
            if name != partition_name:
                in_names.append(name)
        elif alloc.kind == "ExternalOutput":
            out_names.append(name)
            shape = tuple(alloc.tensor_shape)
            dtype = mybir.dt.np(alloc.dtype)
            out_avals.append(jax.core.ShapedArray(shape, dtype))
            zero_outs.append(np.zeros(shape, dtype))
    n_params = len(in_names)
    n_outs = len(out_avals)
    all_names = in_names + out_names
    if partition_name is not None:
        all_names = all_names + [partition_name]

    def _body(*args):
        operands = list(args)
        if partition_name is not None:
            operands.append(bass2jax.partition_id_tensor())
        outs = bass2jax._bass_exec_p.bind(
            *operands,
            out_avals=tuple(out_avals),
            in_names=tuple(all_names),
            out_names=tuple(out_names),
            lowering_input_output_aliases=(),
            sim_require_finite=True,
            sim_require_nnan=True,
            nc=nc,
        )
        return tuple(outs)

    devices = jax.devices()[:N_CORES]
    mesh = Mesh(np.asarray(devices), ("core",))
    in_specs = (PartitionSpec("core"),) * (n_params + n_outs)
    out_specs = (PartitionSpec("core"),) * n_outs
    donate = tuple(range(n_params, n_params + n_outs))
    sharded = jax.jit(
        shard_map(_body, mesh=mesh, in_specs=in_specs, out_specs=out_specs,
                  check_rep=False),
        donate_argnums=donate, keep_unused=True)
    sharded_nodonate = jax.jit(
        shard_map(_body, mesh=mesh, in_specs=in_specs, out_specs=out_specs,
                  check_rep=False),
        keep_unused=True)
    return (sharded, sharded_nodonate, in_names, out_names, zero_outs, mesh)


def _concat_inputs(in_maps, in_names):
    return [np.concatenate([m[name] for m in in_maps], axis=0)
            for name in in_names]


def kernel(x, attn_mask, w_qkv, b_qkv, w_out, b_out):
    global LAST_EXEC_NS
    sharded, _, in_names, out_names, zero_outs, _ = _get_runner()
    in_maps = _prep_inputs(
        np.asarray(x, np.float32), np.asarray(attn_mask),
        np.asarray(w_qkv, np.float32), np.asarray(b_qkv, np.float32),
        np.asarray(w_out, np.float32), np.asarray(b_out, np.float32))
    concat_in = _concat_inputs(in_maps, in_names)
    concat_zeros = [np.zeros((N_CORES * z.shape[0], *z.shape[1:]), z.dtype)
                    for z in zero_outs]
    t0 = time.time()
    out_arrs = sharded(*concat_in, *concat_zeros)
    res = np.asarray(out_arrs[0]).reshape(N_CORES, 2048, 2048)
    t1 = time.time()
    LAST_EXEC_NS = int((t1 - t0) * 1e9)
    bo = np.asarray(b_out, np.float32)
    out = np.empty((4, 2048, 2048), np.float32)
    for b in range(4):
        out[b] = res[2*b] + res[2*b+1] + bo[None, :]
    return out



# revision 29
# speedup vs baseline: 1.0060x; 1.0060x over previous
import sys, os, time
sys.path.insert(0, '/opt/trn_rl_repo')
os.environ.setdefault("JAX_PLATFORMS", "")

import numpy as np
import ml_dtypes

import concourse.bass as bass
import concourse.bacc as bacc
import concourse.bass_isa as bass_isa
import concourse.mybir as mybir
import concourse.tile as tile
from concourse import bass2jax

BF = ml_dtypes.bfloat16
B, S, D, H, DH = 4, 2048, 2048, 16, 128
INV_SQRT_DH = 1.0 / np.sqrt(128.0)

_CACHE = {}
LAST_EXEC_NS = None


def _build(stages="ABC", reps=1, loop_n=1):
    nc = bacc.Bacc("TRN2", target_bir_lowering=False, debug=False, num_devices=8)
    f32, bf16 = mybir.dt.float32, mybir.dt.bfloat16
    x_ap = nc.dram_tensor("x_img", (128, 32768), bf16, kind="ExternalInput").ap()
    wqk_ap = nc.dram_tensor("wqk_img", (128, 32768), bf16, kind="ExternalInput").ap()
    wv_ap = nc.dram_tensor("wv_img", (128, 16384), bf16, kind="ExternalInput").ap()
    wout_ap = nc.dram_tensor("wout_img", (128, 16384), bf16, kind="ExternalInput").ap()
    bqk_ap = nc.dram_tensor("bqk", (128, 16), f32, kind="ExternalInput").ap()
    bv_ap = nc.dram_tensor("bias_v", (128, 1024), f32, kind="ExternalInput").ap()
    mask_ap = nc.dram_tensor("masks", (128, 2048), bf16, kind="ExternalInput").ap()
    onc_ap = nc.dram_tensor("ones_col", (128, 1), bf16, kind="ExternalInput").ap()
    onr_ap = nc.dram_tensor("ones_row", (1, 128), bf16, kind="ExternalInput").ap()
    out_ap = nc.dram_tensor("out", (2048, 2048), f32, kind="ExternalOutput").ap()

    with tile.TileContext(nc) as tc:
        with tc.tile_pool(name="persist", bufs=1) as pp:
            qk_sb = pp.tile([128, 32768], bf16)    # [dh, ob*2048+s], ob=2hl: Q_h^T, 2hl+1: K_h^T
            v_sb = pp.tile([128, 16384], bf16)     # [s%128, st*1024 + hl*128 + dh]
            bqk_sb = pp.tile([128, 16], f32)
            bv_sb = pp.tile([128, 1024], f32)
            ones_c = pp.tile([128, 1], bf16)
            ones_r = pp.tile([1, 128], bf16)
            nc.sync.dma_start(bqk_sb, bqk_ap)
            nc.sync.dma_start(bv_sb, bv_ap)
            nc.sync.dma_start(ones_c, onc_ap)
            nc.sync.dma_start(ones_r, onr_ap)

            def _kernel_body():
                # ---------------- Stage A: QKV projections ----------------
                with tc.tile_pool(name="xTp", bufs=1) as xTp:
                    xT = xTp.tile([128, 32768], bf16)  # [d%128, dt*2048 + s]
                    for dt in range(16 if "A" in stages else 0):
                        nc.sync.dma_start(xT[:, dt*2048:(dt+1)*2048],
                                          x_ap[:, dt*2048:(dt+1)*2048])

                    # A-V first: v_sb = x @ Wv^T + bv   (psum[s, ov])
                    with (
                        tc.tile_pool(name="wvp", bufs=1) as wvp,
                        tc.tile_pool(name="psV", bufs=2, space="PSUM") as psV,
                    ):
                        for oc in range(2 if "A" in stages else 0):
                            wv_oc = wvp.tile([128, 8192], bf16)  # [d%128, dt*512 + ov%512]
                            for dt in range(16):
                                nc.sync.dma_start(
                                    wv_oc[:, dt*512:(dt+1)*512],
                                    wv_ap[:, dt*1024 + oc*512: dt*1024 + (oc+1)*512])
                            for st in range(16):
                                ps = psV.tile([128, 512], f32)
                                for dt in range(16):
                                    nc.tensor.matmul(
                                        ps,
                                        xT[:, dt*2048 + st*128: dt*2048 + (st+1)*128],
                                        wv_oc[:, dt*512:(dt+1)*512],
                                        start=(dt == 0), stop=(dt == 15))
                                nc.vector.tensor_tensor(
                                    v_sb[:, st*1024 + oc*512: st*1024 + (oc+1)*512],
                                    ps, bv_sb[:, oc*512:(oc+1)*512], mybir.AluOpType.add)

                    # A-QK: qk_sb = Wqk @ x^T + b      (psum[o, s])
                    # paired [128,1024] psum tiles, bias-add on ScalarE
                    # sequential accumulation groups (one psum group in
                    # flight at a time, like the pure-MM probe that paces at
                    # 263 ns/MM) — interleaved groups trigger psum-queue
                    # depth-cycling; bias-add on ScalarE per [128,1024] pair.
                    with (
                        tc.tile_pool(name="wqkp", bufs=2) as wqkp,
                        tc.tile_pool(name="psQ", bufs=2, space="PSUM") as psQ,
                    ):
                        for ob in range(16 if "A" in stages else 0):
                            slab = wqkp.tile([128, 2048], bf16)  # [d%128, dt*128 + o]
                            nc.sync.dma_start(slab, wqk_ap[:, ob*2048:(ob+1)*2048])
                            for sp in range(2):
                                ps = psQ.tile([128, 1024], f32)
                                for h2 in range(2):
                                    sc = 2*sp + h2
                                    for dt in range(16):
                                        nc.tensor.matmul(
                                            ps[:, h2*512:(h2+1)*512],
                                            slab[:, dt*128:(dt+1)*128],
                                            xT[:, dt*2048 + sc*512: dt*2048 + (sc+1)*512],
                                            start=(dt == 0), stop=(dt == 15))
                                nc.scalar.activation(
                                    qk_sb[:, ob*2048 + sp*1024: ob*2048 + (sp+1)*1024],
                                    ps, mybir.ActivationFunctionType.Identity,
                                    bias=bqk_sb[:, ob:ob+1])

                # ---------------- Stage B: attention ----------------
                with tc.tile_pool(name="bstat", bufs=1) as bstat:
                    masks = bstat.tile([128, 2048], bf16)
                    ctxT = bstat.tile([128, 16384], bf16)  # [dh, hl*2048 + q]
                    wout = bstat.tile([128, 16384], bf16)  # [dh, hl*2048 + od]
                    if "B" in stages:
                        nc.sync.dma_start(masks, mask_ap)
                    for i in range(8 if "C" in stages else 0):
                        nc.sync.dma_start(wout[:, i*2048:(i+1)*2048],
                                          wout_ap[:, i*2048:(i+1)*2048])

                    with (
                        tc.tile_pool(name="expp", bufs=6) as expp,
                        tc.tile_pool(name="accp", bufs=2) as accp,
                        tc.tile_pool(name="recp", bufs=2) as recp,
                        tc.tile_pool(name="bcp", bufs=2) as bcp,
                        tc.tile_pool(name="psS", bufs=2, space="PSUM") as psS,
                        tc.tile_pool(name="psC", bufs=2, space="PSUM") as psC,
                        tc.tile_pool(name="psD", bufs=1, space="PSUM") as psD,
                        tc.tile_pool(name="psB", bufs=1, space="PSUM") as psB,
                    ):
                        # normalization tail of group g is deferred into group
                        # g+1's score stream so the PE never waits on the
                        # DVE/ACT round-trip (dn -> recip -> bc -> mult).
                        def make_tail(hl, ic, cp, acc_bf):
                            def tail():
                                dn = psD.tile([1, 512], f32, name="dn")
                                for h2 in range(2):
                                    nc.tensor.matmul(
                                        dn, ones_c, acc_bf[:, h2*512:(h2+1)*512],
                                        start=(h2 == 0), stop=(h2 == 1),
                                        skip_group_check=True)
                                rec = recp.tile([1, 512], bf16, name="rec")
                                with nc.allow_low_precision(
                                        reason="softmax denom recip in bf16"):
                                    nc.vector.reciprocal(rec, dn)
                                bc = psB.tile([128, 512], f32, name="bc")
                                nc.tensor.matmul(bc, ones_r, rec, start=True,
                                                 stop=True, skip_group_check=True)
                                bc_sb = bcp.tile([128, 512], f32, name="bc_sb")
                                nc.scalar.activation(
                                    bc_sb, bc, mybir.ActivationFunctionType.Copy)
                                nc.vector.tensor_tensor(
                                    ctxT[:, hl*2048 + ic*512: hl*2048 + (ic+1)*512],
                                    cp, bc_sb, mybir.AluOpType.mult)
                            return tail

                        tail_prev = None
                        for hl in range(8 if "B" in stages else 0):
                            qb = (2*hl) * 2048
                            kb = (2*hl+1) * 2048
                            for ic in range(4):
                                njt = 4*ic + 4
                                njp = njt // 2
                                cp = psC.tile([128, 512], f32)
                                acc = accp.tile([128, 1024], bf16)
                                pend = []

                                def flush_ctx():
                                    jp_, ex_ = pend.pop(0)
                                    for h2 in range(2):
                                        jt_ = 2*jp_ + h2
                                        nc.tensor.matmul(
                                            cp,
                                            v_sb[:, jt_*1024 + hl*128: jt_*1024 + (hl+1)*128],
                                            ex_[:, h2*512:(h2+1)*512],
                                            start=(jt_ == 0), stop=(jt_ == njt-1),
                                            skip_group_check=True)

                                for jp in range(njp):
                                    sc = psS.tile([128, 1024], f32)
                                    for h2 in range(2):
                                        jt = 2*jp + h2
                                        nc.tensor.matmul(
                                            sc[:, h2*512:(h2+1)*512],
                                            qk_sb[:, kb + jt*128: kb + (jt+1)*128],
                                            qk_sb[:, qb + ic*512: qb + (ic+1)*512],
                                            start=True, stop=True, skip_group_check=True)
                                    ex = expp.tile([128, 1024], bf16)
                                    nc.scalar.activation(ex, sc,
                                                         mybir.ActivationFunctionType.Exp)
                                    # causal mask applied AFTER exp as a 0/1
                                    # multiply on the SBUF tile, so the psum
                                    # slot recycles at exp time, off the
                                    # score-stream critical chain.
                                    t0 = 2*jp - 4*ic
                                    if t0 >= 0:
                                        nc.vector.tensor_tensor(
                                            ex, ex, masks[:, t0*512:(t0+2)*512],
                                            mybir.AluOpType.mult)
                                    if jp == 0:
                                        nc.vector.tensor_copy(acc, ex)
                                    else:
                                        nc.vector.tensor_tensor(acc, acc, ex,
                                                                mybir.AluOpType.add)
                                    pend.append((jp, ex))
                                    if jp == 0 and tail_prev is not None:
                                        tail_prev()
                                        tail_prev = None
                                    if len(pend) > 3:
                                        flush_ctx()
                                while pend:
                                    flush_ctx()
                                tail_prev = make_tail(hl, ic, cp, acc)
                        if tail_prev is not None:
                            tail_prev()
                            tail_prev = None

                    # ---------------- Stage C: output projection ----------------
                    # sequential accumulation groups, paired [128,1024] psum
                    # tiles; copies alternate VectorE/ScalarE.
                    with (
                        tc.tile_pool(name="psO", bufs=2, space="PSUM") as psO,
                        tc.tile_pool(name="outp", bufs=3) as outp,
                    ):
                        for st in range(16 if "C" in stages else 0):
                            for dp in range(2):
                                po = psO.tile([128, 1024], f32)
                                for h2 in range(2):
                                    dok = 2*dp + h2
                                    for hl in range(8):
                                        nc.tensor.matmul(
                                            po[:, h2*512:(h2+1)*512],
                                            ctxT[:, hl*2048 + st*128: hl*2048 + (st+1)*128],
                                            wout[:, hl*2048 + dok*512: hl*2048 + (dok+1)*512],
                                            start=(hl == 0), stop=(hl == 7))
                                ob_sb = outp.tile([128, 1024], f32)
                                if (2*st + dp) % 2 == 0:
                                    nc.vector.tensor_copy(ob_sb, po)
                                else:
                                    nc.scalar.activation(
                                        ob_sb, po, mybir.ActivationFunctionType.Copy)
                                nc.sync.dma_start(
                                    out_ap[st*128:(st+1)*128, dp*1024:(dp+1)*1024],
                                    ob_sb)

            if loop_n > 1:
                with tc.For_i(0, loop_n, 1):
                    for _rep in range(reps):
                        _kernel_body()
            else:
                for _rep in range(reps):
                    _kernel_body()

    nc.compile()
    return nc


def _prep_inputs(x, attn_mask, w_qkv, b_qkv, w_out, b_out):
    isd = INV_SQRT_DH
    x_imgs = []
    for b in range(4):
        x_imgs.append(np.ascontiguousarray(
            x[b].reshape(2048, 16, 128).transpose(2, 1, 0).reshape(128, 32768)
        ).astype(BF))

    jj = np.arange(128)[:, None]
    ii = np.arange(512)[None, :]
    mblocks = []
    for t in range(4):
        keep = attn_mask[ii, t*128 + jj] != 0
        mblocks.append(np.where(keep, 1.0, 0.0))
    masks = np.concatenate(mblocks, axis=1).astype(BF)

    ones_col = np.ones((128, 1), dtype=BF)
    ones_row = np.ones((1, 128), dtype=BF)

    per_g = []
    for g in range(2):
        rows = np.empty((2048, 2048), np.float32)
        bqk = np.empty((128, 16), np.float32)
        wv_rows = np.empty((1024, 2048), np.float32)
        bv = np.empty(1024, np.float32)
        for hl in range(8):
            h = 8*g + hl
            rows[(2*hl)*128:(2*hl+1)*128] = w_qkv[384*h: 384*h+128] * isd
            rows[(2*hl+1)*128:(2*hl+2)*128] = w_qkv[384*h+128: 384*h+256]
            bqk[:, 2*hl] = b_qkv[384*h: 384*h+128] * isd
            bqk[:, 2*hl+1] = b_qkv[384*h+128: 384*h+256]
            wv_rows[hl*128:(hl+1)*128] = w_qkv[384*h+256: 384*h+384]
            bv[hl*128:(hl+1)*128] = b_qkv[384*h+256: 384*h+384]
        wqk_img = np.ascontiguousarray(
            rows.reshape(16, 128, 16, 128).transpose(3, 0, 2, 1).reshape(128, 32768)
        ).astype(BF)
        wv_img = np.ascontiguousarray(
            wv_rows.reshape(1024, 16, 128).transpose(2, 1, 0).reshape(128, 16384)
        ).astype(BF)
        wout_img = np.ascontiguousarray(
            w_out[:, 1024*g: 1024*(g+1)].reshape(2048, 8, 128)
            .transpose(2, 1, 0).reshape(128, 16384)
        ).astype(BF)
        bias_v = np.ascontiguousarray(
            np.broadcast_to(bv[None, :], (128, 1024))).astype(np.float32)
        per_g.append((wqk_img, wv_img, wout_img, bqk, bias_v))

    in_maps = []
    for c in range(8):
        b, g = c // 2, c % 2
        wqk_img, wv_img, wout_img, bqk, bias_v = per_g[g]
        in_maps.append({
            "x_img": x_imgs[b],
            "wqk_img": wqk_img,
            "wv_img": wv_img,
            "wout_img": wout_img,
            "bqk": bqk,
            "bias_v": bias_v,
            "masks": masks,
            "ones_col": ones_col,
            "ones_row": ones_row,
        })
    return in_maps


N_CORES = 8


def _get_runner():
    """Build nc + jitted shard_map executable once per process."""
    if "runner" in _CACHE:
        return _CACHE["runner"]
    _CACHE["runner"] = _make_runner(_build())
    return _CACHE["runner"]


def _make_runner(nc):
    import jax
    from jax.sharding import Mesh, PartitionSpec
    from jax.experimental.shard_map import shard_map

    bass2jax.install_neuronx_cc_hook()

    partition_name = (nc.partition_id_tensor.name
                      if nc.partition_id_tensor else None)
    in_names, out_names, out_avals, zero_outs = [], [], [], []
    for alloc in nc.m.functions[0].allocations:
        if not isinstance(alloc, mybir.MemoryLocationSet):
            continue
        name = alloc.memorylocations[0].name
        if alloc.kind == "ExternalInput":
            if name != partition_name:
                in_names.append(name)
        elif alloc.kind == "ExternalOutput":
            out_names.append(name)
            shape = tuple(alloc.tensor_shape)
            dtype = mybir.dt.np(alloc.dtype)
            out_avals.append(jax.core.ShapedArray(shape, dtype))
            zero_outs.append(np.zeros(shape, dtype))
    n_params = len(in_names)
    n_outs = len(out_avals)
    all_names = in_names + out_names
    if partition_name is not None:
        all_names = all_names + [partition_name]

    def _body(*args):
        operands = list(args)
        if partition_name is not None:
            operands.append(bass2jax.partition_id_tensor())
        outs = bass2jax._bass_exec_p.bind(
            *operands,
            out_avals=tuple(out_avals),
            in_names=tuple(all_names),
            out_names=tuple(out_names),
            lowering_input_output_aliases=(),
            sim_require_finite=True,
            sim_require_nnan=True,
            nc=nc,
        )
        return tuple(outs)

    devices = jax.devices()[:N_CORES]
    mesh = Mesh(np.asarray(devices), ("core",))
    in_specs = (PartitionSpec("core"),) * (n_params + n_outs)
    out_specs = (PartitionSpec("core"),) * n_outs
    donate = tuple(range(n_params, n_params + n_outs))
    sharded = jax.jit(
        shard_map(_body, mesh=mesh, in_specs=in_specs, out_specs=out_specs,
                  check_rep=False),
        donate_argnums=donate, keep_unused=True)
    sharded_nodonate = jax.jit(
        shard_map(_body, mesh=mesh, in_specs=in_specs, out_specs=out_specs,
                  check_rep=False),
        keep_unused=True)
    return (sharded, sharded_nodonate, in_names, out_names, zero_outs, mesh)


def _concat_inputs(in_maps, in_names):
    return [np.concatenate([m[name] for m in in_maps], axis=0)
            for name in in_names]


def kernel(x, attn_mask, w_qkv, b_qkv, w_out, b_out):
    global LAST_EXEC_NS
    sharded, _, in_names, out_names, zero_outs, _ = _get_runner()
    in_maps = _prep_inputs(
        np.asarray(x, np.float32), np.asarray(attn_mask),
        np.asarray(w_qkv, np.float32), np.asarray(b_qkv, np.float32),
        np.asarray(w_out, np.float32), np.asarray(b_out, np.float32))
    concat_in = _concat_inputs(in_maps, in_names)
    concat_zeros = [np.zeros((N_CORES * z.shape[0], *z.shape[1:]), z.dtype)
                    for z in zero_outs]
    t0 = time.time()
    out_arrs = sharded(*concat_in, *concat_zeros)
    res = np.asarray(out_arrs[0]).reshape(N_CORES, 2048, 2048)
    t1 = time.time()
    LAST_EXEC_NS = int((t1 - t0) * 1e9)
    bo = np.asarray(b_out, np.float32)
    out = np.empty((4, 2048, 2048), np.float32)
    for b in range(4):
        out[b] = res[2*b] + res[2*b+1] + bo[None, :]
    return out

